# revision 6
# baseline (speedup 1.0000x reference)
"""Trainium2 Bass kernel for nn_ClipCluLoss (clip-cluster loss).

Math (collapsed form of the reference):
    w[b,t]  = 1 / max(||x[b,t,:]||_2, 1e-12)
    s[b,d]  = sum_t w[b,t] * x[b,t,d]          (= T * mean_rep[b,d])
    loss    = T - (1/(B*T)) * sum_b ||s[b]||^2

Sharding: data-parallel over B across 8 NeuronCores (128 samples/core).
Each core returns its partial sum_b ||s_b||^2 as a [1,1] tensor; the host
does the scalar epilogue.

Per-core structure (x viewed as [4096 rows=(b,t), 1024 d], 32 chunks of
128 rows, DMA'd in 16 pairs of 1 MiB):
  per pair pr (chunks k=2pr, 2pr+1):
    SP   : DMA pair -> xt[pr%NBUF] [128, 2048]
    DVE  : ss_d[:,c] = sum_d x_{h=0}^2       (scalar_tensor_tensor fused)
    ACT  : ss_a[:,c] = sum_d x_{h=1}^2       (Square + accum_out in PSUM)
    ACT  : wp = sqrt(ss)   (two cols)
    DVE  : wp = 1/max(wp, eps)
    POOL : A[k%NA][:, 4k+j] = mask01 * wp    (block-sparse lhsT build)
    PE   : S0 += A^T x[:, h, :512] ; S1 += A^T x[:, h, 512:]  (PSUM accum)
  epilogue: DVE copies S to SBUF, fused square+reduce -> q[128,1],
            PE ones-matmul partition-reduce -> [1,1] -> DMA out.

Raw Bass (manual semaphores) because this container's walrus rejects
Tile-generated multi-wait sync and the TENSOR_TENSOR_REDUCE ISA op.
"""

import sys
from contextlib import ExitStack

import numpy as np

for _p in ("/opt/trn_rl_repo",):
    if _p not in sys.path:
        sys.path.insert(0, _p)

import concourse.bass as bass
from concourse import mybir
from concourse.bass_utils import run_bass_kernel_spmd

B, T, D = 1024, 32, 1024
N_CORES = 8
BS = B // N_CORES            # samples per core
P = 128                      # SBUF partitions
ROWS = BS * T                # 4096 rows of (b,t) per core
NCHUNK = ROWS // P           # 32 chunks of 128 rows
PAIRS = NCHUNK // 2          # 2 chunks per DMA (1 MiB each)
EPS = 1e-12

NBUF = 4                     # x input double-buffers (pairs)
NS = 4                       # ss/wp rotation depth (pairs)
NA = 4                       # A (lhsT) buffers (chunks)

F32 = mybir.dt.float32
ALU = mybir.AluOpType
ACTF = mybir.ActivationFunctionType


def build_bass() -> bass.Bass:
    nc = bass.Bass(trn_type="TRN2")
    x_h = nc.declare_dram_parameter("x", [BS, T, D], F32, isOutput=False)
    out_h = nc.declare_dram_parameter("out", [1, 1], F32, isOutput=True)
    x_flat = x_h[:, :, :].flatten_outer_dims()      # [4096, 1024]

    ctx = ExitStack()
    with ctx:
        xt = [
            ctx.enter_context(nc.sbuf_tensor(f"xt{i}", [P, 2 * D], F32))
            for i in range(NBUF)
        ]
        a_t = [
            ctx.enter_context(nc.sbuf_tensor(f"a_t{i}", [P, P], F32))
            for i in range(NA)
        ]
        scr_d = ctx.enter_context(nc.sbuf_tensor([P, D], F32))   # DVE STT out
        scr_a = ctx.enter_context(nc.sbuf_tensor([P, D], F32))   # ACT Square out
        ss_d = ctx.enter_context(nc.sbuf_tensor([P, NS], F32))   # DVE accum (SBUF)
        wp = [
            ctx.enter_context(nc.sbuf_tensor(f"wp{i}", [P, 2], F32))
            for i in range(NS)
        ]
        mask01 = ctx.enter_context(nc.sbuf_tensor([P, 4], F32))
        ones = ctx.enter_context(nc.sbuf_tensor([P, 1], F32))
        qs = ctx.enter_context(nc.sbuf_tensor([P, 1], F32))
        outsb = ctx.enter_context(nc.sbuf_tensor([1, 1], F32))

        s0 = ctx.enter_context(nc.psum_tensor([P, 512], F32))
        s1 = ctx.enter_context(nc.psum_tensor([P, 512], F32))
        ss_a = ctx.enter_context(nc.psum_tensor([P, NS], F32))   # ACT accum (PSUM)
        qps = ctx.enter_context(nc.psum_tensor([1, 1], F32))

        dma_sem = ctx.enter_context(nc.semaphore("dma_sem"))
        ss_sem = ctx.enter_context(nc.semaphore("ss_sem"))      # DVE STT per pair
        sqrt_sem = ctx.enter_context(nc.semaphore("sqrt_sem"))  # ACT sqrt per pair
        w_sem = ctx.enter_context(nc.semaphore("w_sem"))        # DVE recip per pair
        a_sem = ctx.enter_context(nc.semaphore("a_sem"))        # POOL mask per chunk
        mm_sem = ctx.enter_context(nc.semaphore("mm_sem"))      # PE per chunk
        fin_sem = ctx.enter_context(nc.semaphore("fin_sem"))
        block = ctx.enter_context(nc.Block())

        @block.sync
        def _(sp):
            for pr in range(PAIRS):
                if pr >= NBUF:
                    sp.wait_ge(mm_sem, 2 * (pr - NBUF) + 2)
                src = x_flat[256 * pr : 256 * (pr + 1), :].rearrange(
                    "(h p) d -> p h d", p=P
                )
                dst = xt[pr % NBUF][:, :].rearrange("p (h d) -> p h d", h=2)
                sp.dma_start(out=dst, in_=src).then_inc(dma_sem, 16)
            sp.wait_ge(fin_sem, 3)
            sp.dma_start(out=out_h[:, :], in_=outsb[:, :]).then_inc(dma_sem, 16)

        @block.vector
        def _(v):
            def wchain(pr):
                c = pr % NS
                v.wait_ge(sqrt_sem, pr + 1)
                v.tensor_scalar_max(out=wp[c][:, :], in0=wp[c][:, :], scalar1=EPS)
                v.reciprocal(out=wp[c][:, :], in_=wp[c][:, :]).then_inc(w_sem, 1)

            for pr in range(PAIRS):
                v.wait_ge(dma_sem, 16 * (pr + 1))
                v.scalar_tensor_tensor(
                    out=scr_d[:, :],
                    in0=xt[pr % NBUF][:, 0:D],
                    scalar=1.0,
                    in1=xt[pr % NBUF][:, 0:D],
                    op0=ALU.mult,
                    op1=ALU.mult,
                    accum_out=ss_d[:, pr % NS : pr % NS + 1],
                ).then_inc(ss_sem, 1)
                if pr >= 1:
                    wchain(pr - 1)
            wchain(PAIRS - 1)

            # epilogue: q[p] = sum_f S[p, f]^2
            v.wait_ge(mm_sem, NCHUNK)
            v.tensor_copy(out=scr_d[:, 0:512], in_=s0[:, :])
            v.tensor_copy(out=scr_d[:, 512:1024], in_=s1[:, :])
            v.scalar_tensor_tensor(
                out=scr_a[:, :],
                in0=scr_d[:, :],
                scalar=1.0,
                in1=scr_d[:, :],
                op0=ALU.mult,
                op1=ALU.mult,
                accum_out=qs[:, :],
            ).then_inc(fin_sem, 1)
            v.wait_ge(fin_sem, 2)  # PE partition-reduce done
            v.tensor_copy(out=outsb[:, :], in_=qps[:, :]).then_inc(fin_sem, 1)

        @block.scalar
        def _(s):
            def sqrtstep(pr):
                c = pr % NS
                s.wait_ge(ss_sem, pr + 1)
                if pr >= NS:
                    s.wait_ge(a_sem, 2 * (pr - NS) + 2)  # WAR: wp[c] readers done
                s.sqrt(out=wp[c][:, 0:1], in_=ss_d[:, c : c + 1])
                s.sqrt(out=wp[c][:, 1:2], in_=ss_a[:, c : c + 1]).then_inc(sqrt_sem, 1)

            for pr in range(PAIRS):
                s.wait_ge(dma_sem, 16 * (pr + 1))
                s.activation(
                    out=scr_a[:, :],
                    in_=xt[pr % NBUF][:, D : 2 * D],
                    func=ACTF.Square,
                    accum_out=ss_a[:, pr % NS : pr % NS + 1],
                )
                if pr >= 1:
                    sqrtstep(pr - 1)
            sqrtstep(PAIRS - 1)

        @block.gpsimd
        def _(g):
            for i in range(NA):
                g.memset(a_t[i][:, :], 0.0)
            g.memset(mask01[:, :], 0.0)
            for j in range(4):
                g.memset(mask01[32 * j : 32 * (j + 1), j : j + 1], 1.0)
            g.memset(ones[:, :], 1.0)
            for k in range(NCHUNK):
                pr, h = k // 2, k % 2
                if h == 0:
                    g.wait_ge(w_sem, pr + 1)
                if k >= NA:
                    g.wait_ge(mm_sem, k - NA + 1)
                    g.memset(a_t[k % NA][:, 4 * (k - NA) : 4 * (k - NA) + 4], 0.0)
                g.tensor_scalar_mul(
                    out=a_t[k % NA][:, 4 * k : 4 * k + 4],
                    in0=mask01[:, :],
                    scalar1=wp[pr % NS][:, h : h + 1],
                ).then_inc(a_sem, 1)

        @block.tensor
        def _(t):
            for k in range(NCHUNK):
                pr, h = k // 2, k % 2
                t.wait_ge(a_sem, k + 1)
                start = k == 0
                stop = k == NCHUNK - 1
                t.matmul(
                    s0[:, :],
                    a_t[k % NA][:, :],
                    xt[pr % NBUF][:, D * h : D * h + 512],
                    start=start,
                    stop=stop,
                )
                t.matmul(
                    s1[:, :],
                    a_t[k % NA][:, :],
                    xt[pr % NBUF][:, D * h + 512 : D * h + 1024],
                    start=start,
                    stop=stop,
                ).then_inc(mm_sem, 1)
            t.wait_ge(fin_sem, 1)
            t.matmul(qps[:, :], ones[:, :], qs[:, :], start=True, stop=True).then_inc(
                fin_sem, 1
            )

    return nc


_NC_CACHE: dict = {}


def _get_nc() -> bass.Bass:
    if "nc" not in _NC_CACHE:
        _NC_CACHE["nc"] = build_bass()
    return _NC_CACHE["nc"]


def run_cores(x: np.ndarray, **spmd_kwargs):
    """Run the SPMD kernel on 8 cores. Returns (partials, BassKernelResults)."""
    nc = _get_nc()
    in_maps = [
        {"x": np.ascontiguousarray(x[c * BS : (c + 1) * BS])}
        for c in range(N_CORES)
    ]
    res = run_bass_kernel_spmd(nc, in_maps, core_ids=list(range(N_CORES)),
                               **spmd_kwargs)
    partials = [float(r["out"][0, 0]) for r in res.results]
    return partials, res


def kernel(inputs: np.ndarray) -> np.ndarray:
    x = np.ascontiguousarray(np.asarray(inputs, dtype=np.float32))
    assert x.shape == (B, T, D), x.shape
    partials, _ = run_cores(x)
    loss = np.float64(T) - np.float64(sum(partials)) / (B * T)
    return np.array(loss, dtype=np.float32)


# revision 13
# speedup vs baseline: 1.2996x; 1.2996x over previous
"""Trainium2 Bass kernel for nn_ClipCluLoss (clip-cluster loss).

Math (collapsed form of the reference):
    w[b,t]  = 1 / max(||x[b,t,:]||_2, 1e-12)
    s[b,d]  = sum_t w[b,t] * x[b,t,d]          (= T * mean_rep[b,d])
    loss    = T - (1/(B*T)) * sum_b ||s[b]||^2

Sharding: data-parallel over B across 8 NeuronCores (128 samples/core).
Each core returns its partial sum_b ||s_b||^2 as a [1,1] tensor; the host
does the scalar epilogue.

Per-core structure (x viewed as [4096 rows=(b,t), 1024 d], 32 chunks of
128 rows; whole bf16 shard resident in SBUF, all input DMAs issued
up-front):
  POOL : SWDGE cast-DMA f32 HBM -> bf16 SBUF (fp32 matmul on TRN2 runs
         as a 2-pass LOW_HIGH emulation, ~4x slower than bf16 - so the
         matmul path is bf16; norms and all accumulation stay f32)
  DVE  : ss = sum_d x^2 for 2 chunks/quad (fused scalar_tensor_tensor)
  ACT  : ss for the other 2 chunks/quad (Square + accum_out in PSUM)
  ACT  : wp = sqrt(ss);  DVE: wp = 1/max(wp, eps)
  POOL : A[k%NA][:, 4k+j] = mask01 * wp   (block-sparse bf16 lhsT)
  PE   : S0 += A^T x[:, :512] ; S1 += A^T x[:, 512:]   (f32 PSUM accum)
  epilogue: DVE copies S to SBUF, fused square+reduce -> q[128,1],
            PE ones-matmul partition-reduce -> [1,1] -> DMA out (SP).

Raw Bass (manual semaphores): this container's walrus rejects
Tile-generated multi-wait sync and the TENSOR_TENSOR_REDUCE ISA op.
Each input DMA gets its own semaphore: a shared counter with +16 per DMA
is NOT completion-ordered across DMAs (16 SDMA engines increment
independently), which produced data races under 8-core HBM contention.
"""

import sys
from contextlib import ExitStack

import numpy as np

for _p in ("/opt/trn_rl_repo",):
    if _p not in sys.path:
        sys.path.insert(0, _p)

import concourse.bass as bass
from concourse import mybir
from concourse.bass_utils import run_bass_kernel_spmd

B, T, D = 1024, 32, 1024
N_CORES = 8
BS = B // N_CORES            # samples per core
P = 128                      # SBUF partitions
ROWS = BS * T                # 4096 rows of (b,t) per core
NCHUNK = ROWS // P           # 32 chunks of 128 rows
QUADS = NCHUNK // 4          # 4 chunks per quad
EPS = 1e-12

NS = 4                       # ss/wp rotation depth (quads)
NA = 8                       # A (lhsT) buffers (chunks)

F32 = mybir.dt.float32
BF16 = mybir.dt.bfloat16
ALU = mybir.AluOpType
ACTF = mybir.ActivationFunctionType

# DMA units: (first_chunk, n_chunks). Chunk-granular at head and tail so
# the compute pipeline ramps/drains with ~512 KiB latency, 2 MiB quads
# in the middle. Each unit completes on its own semaphore.
DMA_UNITS = (
    [(h, 1) for h in range(4)]
    + [(4 * q, 4) for q in range(1, QUADS - 1)]
    + [(NCHUNK - 4 + h, 1) for h in range(4)]
)
_CHUNK_UNIT = {}
for _u, (_c0, _n) in enumerate(DMA_UNITS):
    for _c in range(_c0, _c0 + _n):
        _CHUNK_UNIT[_c] = _u
assert len(_CHUNK_UNIT) == NCHUNK


def build_bass(debug: bool = False) -> bass.Bass:
    nc = bass.Bass(trn_type="TRN2")
    x_h = nc.declare_dram_parameter("x", [BS, T, D], F32, isOutput=False)
    out_h = nc.declare_dram_parameter("out", [1, 1], F32, isOutput=True)
    dbg_h = None
    if debug:
        dbg_h = nc.declare_dram_parameter("dbg", [P, 1024 + 32 + 8 * P], F32,
                                          isOutput=True)
    x_flat = x_h[:, :, :].flatten_outer_dims()      # [4096, 1024]

    ctx = ExitStack()
    with ctx:
        xb = [
            ctx.enter_context(nc.sbuf_tensor(f"xb{i}", [P, 4 * D], BF16))
            for i in range(QUADS)
        ]
        a_t = [
            ctx.enter_context(nc.sbuf_tensor(f"a_t{i}", [P, P], BF16))
            for i in range(NA)
        ]
        scr_d = ctx.enter_context(nc.sbuf_tensor("scr_d", [P, D], BF16))
        scr_a = ctx.enter_context(nc.sbuf_tensor("scr_a", [P, D], BF16))
        ss_d = ctx.enter_context(nc.sbuf_tensor("ss_d", [P, 2 * NS], F32))
        wp = [
            ctx.enter_context(nc.sbuf_tensor(f"wp{i}", [P, 4], F32))
            for i in range(NS)
        ]
        mask01 = ctx.enter_context(nc.sbuf_tensor("mask01", [P, 4], BF16))
        ones = ctx.enter_context(nc.sbuf_tensor("ones", [P, 1], F32))
        qs = ctx.enter_context(nc.sbuf_tensor("qs", [P, 1], F32))
        sepi = ctx.enter_context(nc.sbuf_tensor("sepi", [P, D], F32))
        sepo = ctx.enter_context(nc.sbuf_tensor("sepo", [P, D], F32))
        dum = ctx.enter_context(nc.sbuf_tensor("dum", [P, 1], F32))
        outsb = ctx.enter_context(nc.sbuf_tensor("outsb", [1, 1], F32))
        dbg_t = None
        if debug:
            dbg_t = ctx.enter_context(
                nc.sbuf_tensor("dbgt", [P, 1024 + 32 + 8 * P], F32)
            )

        s0 = ctx.enter_context(nc.psum_tensor([P, 512], F32))
        s1 = ctx.enter_context(nc.psum_tensor([P, 512], F32))
        ss_a = ctx.enter_context(nc.psum_tensor([P, 2 * NS], F32))
        qps = ctx.enter_context(nc.psum_tensor([1, 1], F32))

        dsem = [
            ctx.enter_context(nc.semaphore(f"dsem{u}"))
            for u in range(len(DMA_UNITS))
        ]
        odma_sem = ctx.enter_context(nc.semaphore("odma_sem"))
        ss_sem = ctx.enter_context(nc.semaphore("ss_sem"))      # DVE STTs /quad
        sqrt_sem = ctx.enter_context(nc.semaphore("sqrt_sem"))  # ACT sqrt /quad
        w_sem = ctx.enter_context(nc.semaphore("w_sem"))        # DVE recip /quad
        a_sem = ctx.enter_context(nc.semaphore("a_sem"))        # POOL mask /chunk
        mm_sem = ctx.enter_context(nc.semaphore("mm_sem"))      # PE /chunk
        fin_sem = ctx.enter_context(nc.semaphore("fin_sem"))
        block = ctx.enter_context(nc.Block())

        def xb_chunk(k):
            """bf16 SBUF view of chunk k: [128, 1024]."""
            q, h = k // 4, k % 4
            return xb[q][:, D * h : D * (h + 1)]

        def wait_chunk(eng, k):
            eng.wait_ge(dsem[_CHUNK_UNIT[k]], 16)

        @block.gpsimd
        def _(g):
            # all input DMAs up-front; every buffer written exactly once
            for u, (c0, n) in enumerate(DMA_UNITS):
                q = c0 // 4
                src = x_flat[128 * c0 : 128 * (c0 + n), :]
                if n > 1:
                    src = src.rearrange("(h p) d -> p h d", p=P)
                    dst = xb[q][:, :].rearrange("p (h d) -> p h d", h=4)
                else:
                    dst = xb_chunk(c0)
                g.dma_start(out=dst, in_=src).then_inc(dsem[u], 16)

            for i in range(NA):
                g.memset(a_t[i][:, :], 0.0)
            g.memset(mask01[:, :], 0.0)
            for j in range(4):
                g.memset(mask01[32 * j : 32 * (j + 1), j : j + 1], 1.0)
            g.memset(ones[:, :], 1.0)

            for k in range(NCHUNK):
                q, h = k // 4, k % 4
                if h == 0:
                    g.wait_ge(w_sem, q + 1)
                if k >= NA:
                    g.wait_ge(mm_sem, k - NA + 1)
                    g.memset(
                        a_t[k % NA][:, 4 * (k - NA) : 4 * (k - NA) + 4], 0.0
                    )
                g.tensor_scalar_mul(
                    out=a_t[k % NA][:, 4 * k : 4 * k + 4],
                    in0=mask01[:, :],
                    scalar1=wp[q % NS][:, h : h + 1],
                ).then_inc(a_sem, 1)

        @block.vector
        def _(v):
            def wchain(q):
                c = q % NS
                v.wait_ge(sqrt_sem, q + 1)
                v.tensor_scalar_max(out=wp[c][:, :], in0=wp[c][:, :], scalar1=EPS)
                v.reciprocal(out=wp[c][:, :], in_=wp[c][:, :]).then_inc(w_sem, 1)

            for q in range(QUADS):
                for h in (0, 1):
                    k = 4 * q + h
                    wait_chunk(v, k)
                    ins = v.scalar_tensor_tensor(
                        out=scr_d[:, :],
                        in0=xb_chunk(k),
                        scalar=1.0,
                        in1=xb_chunk(k),
                        op0=ALU.mult,
                        op1=ALU.mult,
                        accum_out=ss_d[:, 2 * (q % NS) + h : 2 * (q % NS) + h + 1],
                    )
                    if h == 1:
                        ins.then_inc(ss_sem, 1)
                if q >= 1:
                    wchain(q - 1)
            wchain(QUADS - 1)

            # epilogue: q[p] = sum_f S[p, f]^2
            v.wait_ge(mm_sem, NCHUNK)
            v.tensor_copy(out=sepi[:, 0:512], in_=s0[:, :])
            v.tensor_copy(out=sepi[:, 512:1024], in_=s1[:, :])
            v.scalar_tensor_tensor(
                out=sepo[:, :],
                in0=sepi[:, :],
                scalar=1.0,
                in1=sepi[:, :],
                op0=ALU.mult,
                op1=ALU.mult,
                accum_out=qs[:, :],
            ).then_inc(fin_sem, 1)
            v.wait_ge(fin_sem, 2)  # PE partition-reduce done
            v.tensor_copy(out=outsb[:, :], in_=qps[:, :]).then_inc(fin_sem, 1)
            if debug:
                v.tensor_copy(out=dbg_t[:, 0:1024], in_=sepi[:, :])
                v.tensor_copy(out=dbg_t[:, 1024:1032], in_=ss_d[:, :])
                v.tensor_copy(out=dbg_t[:, 1032:1040], in_=ss_a[:, :])
                for i in range(NS):
                    v.tensor_copy(out=dbg_t[:, 1040 + 4 * i : 1044 + 4 * i],
                                  in_=wp[i][:, :])
                for i in range(NA):
                    ins = v.tensor_copy(
                        out=dbg_t[:, 1056 + P * i : 1056 + P * (i + 1)],
                        in_=a_t[i][:, :],
                    )
                ins.then_inc(fin_sem, 1)

        @block.scalar
        def _(s):
            # trigger the sqrt ACT table load during the first DMA
            s.sqrt(out=dum[:, :], in_=dum[:, :])

            def sqrtstep(q):
                c = q % NS
                s.wait_ge(ss_sem, q + 1)
                if q >= NS:
                    s.wait_ge(a_sem, 4 * (q - NS) + 4)  # WAR: wp[c] readers done
                s.sqrt(out=wp[c][:, 0:2], in_=ss_d[:, 2 * c : 2 * c + 2])
                s.sqrt(out=wp[c][:, 2:4], in_=ss_a[:, 2 * c : 2 * c + 2]).then_inc(
                    sqrt_sem, 1
                )

            for q in range(QUADS):
                for h in (2, 3):
                    k = 4 * q + h
                    wait_chunk(s, k)
                    s.activation(
                        out=scr_a[:, :],
                        in_=xb_chunk(k),
                        func=ACTF.Square,
                        accum_out=ss_a[:, 2 * (q % NS) + h - 2 : 2 * (q % NS) + h - 1],
                    )
                if q >= 1:
                    sqrtstep(q - 1)
            sqrtstep(QUADS - 1)

        @block.tensor
        def _(t):
            for k in range(NCHUNK):
                t.wait_ge(a_sem, k + 1)
                start = k == 0
                stop = k == NCHUNK - 1
                t.matmul(
                    s0[:, :],
                    a_t[k % NA][:, :],
                    xb_chunk(k)[:, 0:512],
                    start=start,
                    stop=stop,
                )
                t.matmul(
                    s1[:, :],
                    a_t[k % NA][:, :],
                    xb_chunk(k)[:, 512:1024],
                    start=start,
                    stop=stop,
                ).then_inc(mm_sem, 1)
            t.wait_ge(fin_sem, 1)
            t.matmul(qps[:, :], ones[:, :], qs[:, :], start=True, stop=True).then_inc(
                fin_sem, 1
            )

        @block.sync
        def _(sp):
            sp.wait_ge(fin_sem, 3)
            sp.dma_start(out=out_h[:, :], in_=outsb[:, :]).then_inc(odma_sem, 16)
            if debug:
                sp.wait_ge(fin_sem, 4)
                sp.dma_start(out=dbg_h[:, :], in_=dbg_t[:, :]).then_inc(
                    odma_sem, 16
                )

    return nc


_NC_CACHE: dict = {}


def _get_nc() -> bass.Bass:
    if "nc" not in _NC_CACHE:
        _NC_CACHE["nc"] = build_bass()
    return _NC_CACHE["nc"]


def run_cores(x: np.ndarray, **spmd_kwargs):
    """Run the SPMD kernel on 8 cores. Returns (partials, BassKernelResults)."""
    nc = _get_nc()
    in_maps = [
        {"x": np.ascontiguousarray(x[c * BS : (c + 1) * BS])}
        for c in range(N_CORES)
    ]
    res = run_bass_kernel_spmd(nc, in_maps, core_ids=list(range(N_CORES)),
                               **spmd_kwargs)
    partials = [float(r["out"][0, 0]) for r in res.results]
    return partials, res


def kernel(inputs: np.ndarray) -> np.ndarray:
    x = np.ascontiguousarray(np.asarray(inputs, dtype=np.float32))
    assert x.shape == (B, T, D), x.shape
    partials, _ = run_cores(x)
    loss = np.float64(T) - np.float64(sum(partials)) / (B * T)
    return np.array(loss, dtype=np.float32)
